# revision 3
# baseline (speedup 1.0000x reference)
"""Trainium2 Bass kernel for nn_CNNMnist_Sketch (sketched CNN forward pass).

Data-parallel over 8 NeuronCores: batch 4096 -> 512 per core.
Per-core pipeline (all shapes hardcoded):
  conv1 5x5 (1->32ch) + maxpool2 + relu   -> h1  [32ch, 12x12]
  conv2 5x5 (32->64ch) + maxpool2 + relu  -> h2  [64ch, 4x4] -> flat 1024
  fc1 1024->512 + relu, fc2 512->10, log_softmax

Key layout tricks:
  - conv1: input replicated to 100 SBUF partitions (4 batch-chunks x 25 taps),
    each partition pre-shifted by its tap offset; a single block-diagonal
    [100,128] lhsT computes 4 chunks x 32 channels in one matmul stream.
  - conv2: pooled h1 replicated to 128 partitions (4 kw-shifted copies x 32ch)
    so 4 taps contract per pass (5 K=128 passes + 5 K=32 passes for kw=4).
  - fc2/log_softmax run with batch on partitions -> free-dim reductions.
"""

import numpy as np
import ml_dtypes

import concourse.bass as bass
import concourse.bacc as bacc
import concourse.tile as tile
from concourse import mybir
from concourse.bass_utils import run_bass_kernel_spmd

F32 = mybir.dt.float32
F32R = mybir.dt.float32r
BF16 = mybir.dt.bfloat16
RELU = mybir.ActivationFunctionType.Relu
EXP = mybir.ActivationFunctionType.Exp
LN = mybir.ActivationFunctionType.Ln
MAXOP = mybir.AluOpType.max
SUBOP = mybir.AluOpType.subtract
ADDOP = mybir.AluOpType.add
AXY = mybir.AxisListType.XY
AX = mybir.AxisListType.X

NCORES = 8
BPC = 4096 // NCORES          # samples per core
BLK = 64                      # samples per block
NBLK = BPC // BLK
CS = BLK // 4                 # samples per conv1 chunk (4 chunks / block)
CHUNKF = CS * 784             # x elements per chunk
XBLK = BLK * 784              # x elements per block
H1F = CS * 144                # h1 elements per chunk (per channel)
XPAD = 128                    # DRAM pad so shifted reads never go OOB

_CACHE = {}


def _build():
    nc = bacc.Bacc(target_bir_lowering=False, debug=False, num_devices=NCORES)

    xt = nc.dram_tensor("x", [BPC * 784 + XPAD], BF16, kind="ExternalInput").ap()
    wc1t = nc.dram_tensor("wc1bd", [100, 128], BF16, kind="ExternalInput").ap()
    w2at = nc.dram_tensor("w2a", [128, 5 * 64], BF16, kind="ExternalInput").ap()
    w2bt = nc.dram_tensor("w2b", [32, 5 * 64], BF16, kind="ExternalInput").ap()
    w3t = nc.dram_tensor("w3sb", [128, 4096], BF16, kind="ExternalInput").ap()
    fc2t = nc.dram_tensor("fc2sb", [128, 40], F32, kind="ExternalInput").ap()
    b1t = nc.dram_tensor("b1r", [128, 1], F32, kind="ExternalInput").ap()
    b2t = nc.dram_tensor("b2", [64, 1], F32, kind="ExternalInput").ap()
    b3t = nc.dram_tensor("b3sb", [128, 4], F32, kind="ExternalInput").ap()
    fbt = nc.dram_tensor("fc2b", [1, 10], F32, kind="ExternalInput").ap()
    ot = nc.dram_tensor("out", [BPC, 10], F32, kind="ExternalOutput").ap()

    from contextlib import ExitStack

    with tile.TileContext(nc, num_cores=NCORES) as tc, ExitStack() as es:
        W = es.enter_context(tc.tile_pool(name="weights", bufs=1))
        S = es.enter_context(tc.tile_pool(name="work", bufs=2))
        P = es.enter_context(tc.tile_pool(name="persist", bufs=1))
        PS = es.enter_context(tc.tile_pool(name="ps", bufs=6, space="PSUM"))

        # ---- load weights ----
        wc1 = W.tile([100, 128], BF16)
        nc.sync.dma_start(out=wc1[:], in_=wc1t)
        w2a = W.tile([128, 320], BF16)
        nc.sync.dma_start(out=w2a[:], in_=w2at)
        w2b = W.tile([32, 320], BF16)
        nc.sync.dma_start(out=w2b[:], in_=w2bt)
        w3 = W.tile([128, 4096], BF16)
        nc.sync.dma_start(out=w3[:], in_=w3t)
        fc2 = W.tile([128, 40], F32)
        nc.sync.dma_start(out=fc2[:], in_=fc2t)
        b1r = W.tile([128, 1], F32)
        nc.sync.dma_start(out=b1r[:], in_=b1t)
        b2 = W.tile([64, 1], F32)
        nc.sync.dma_start(out=b2[:], in_=b2t)
        b3 = W.tile([128, 4], F32)
        nc.sync.dma_start(out=b3[:], in_=b3t)
        fc2b = W.tile([1, 10], F32)
        nc.sync.dma_start(out=fc2b[:], in_=fbt)
        ones1 = W.tile([1, 128], F32)
        nc.vector.memset(ones1[:], 1.0)

        h2 = P.tile([64, 16 * BPC], BF16)          # free = (sp outer, b inner)
        DR = es.enter_context(tc.tile_pool(name="dram", bufs=2, space="DRAM"))

        for blk in range(NBLK):
            xbase = blk * XBLK
            # ---- conv1 input: 2-hop shift-replication -> [100, CHUNKF] ----
            # hop A: partition 5j+kh = chunk j shifted by image-row kh
            # hop B: partition 25j+5kh+kw = hop-A partition shifted by kw
            xrep = S.tile([120, CHUNKF + 8], BF16, tag="xrep")
            srcA = bass.AP(
                tensor=xt.tensor,
                offset=xbase,
                ap=[[CHUNKF, 4], [28, 5], [1, CHUNKF]],
            )
            nc.sync.dma_start(out=xrep[100:120, 0:CHUNKF], in_=srcA)
            srcB = bass.AP(
                tensor=xrep[:].tensor,
                offset=xrep[:].offset + 100 * (CHUNKF + 8),
                ap=[[CHUNKF + 8, 20], [1, 5], [1, CHUNKF]],
            )
            nc.gpsimd.dma_start(out=xrep[0:100, 0:CHUNKF], in_=srcB)

            # ---- conv1 matmuls + pool (bf16 PSUM: one matmul per bank) ----
            h1p = S.tile([128, H1F], BF16, tag="h1p")
            for s in range(CS):
                for h in range(2):
                    ps1 = PS.tile([128, 288], F32, tag="ps")
                    rhs = bass.AP(
                        tensor=xrep[:].tensor,
                        offset=xrep[:].offset + s * 784 + h * 336,
                        ap=[[CHUNKF + 8, 100], [28, 12], [1, 24]],
                    )
                    nc.tensor.matmul(
                        out=ps1[:], lhsT=wc1[:], rhs=rhs, start=True, stop=True
                    )
                    pv = ps1[:].rearrange(
                        "p (ph s1 pw s0) -> p ph pw s1 s0", ph=6, s1=2, pw=12, s0=2
                    )
                    ov = bass.AP(
                        tensor=h1p[:].tensor,
                        offset=h1p[:].offset + s * 144 + h * 72,
                        ap=[[H1F, 128], [12, 6], [1, 12]],
                    )
                    nc.vector.tensor_reduce(out=ov, in_=pv, axis=AXY, op=MAXOP)
            # bias + relu (both commute with maxpool)
            nc.scalar.activation(h1p[:], h1p[:], RELU, bias=b1r[:])

            # ---- conv2 input: kw-shifted 4x replication via DRAM bounce ----
            # write h1p to DRAM permuted [ci, j, f]; read back so partition
            # 32c+ci holds (chunk j at free j*H1F) shifted by c
            h1d = DR.tile([32 * 4 * H1F + 8], BF16, tag="h1d")
            for j in range(4):
                dst = bass.AP(
                    tensor=h1d[:].tensor,
                    offset=h1d[:].offset + j * H1F,
                    ap=[[4 * H1F, 32], [1, H1F]],
                )
                eng = nc.sync if j % 2 == 0 else nc.gpsimd
                eng.dma_start(out=dst, in_=h1p[32 * j : 32 * j + 32, :])
            h1r = S.tile([128, 4 * H1F + 8], BF16, tag="h1r")
            rdsrc = bass.AP(
                tensor=h1d[:].tensor,
                offset=h1d[:].offset,
                ap=[[1, 4], [4 * H1F, 32], [1, 4 * H1F]],
            )
            nc.scalar.dma_start(out=h1r[0:128, 0 : 4 * H1F], in_=rdsrc)

            for g in range(8):            # 8 sample-groups of 8 within block
                j, hh = divmod(g, 2)
                goff = j * H1F + hh * 8 * 144
                ps2 = PS.tile([64, 512], F32, tag="ps")
                for kh in range(5):
                    rhs = bass.AP(
                        tensor=h1r[:].tensor,
                        offset=h1r[:].offset + goff + 12 * kh,
                        ap=[[4 * H1F + 8, 128], [144, 8], [12, 8], [1, 8]],
                    )
                    nc.tensor.matmul(
                        out=ps2[:],
                        lhsT=w2a[:, 64 * kh : 64 * kh + 64],
                        rhs=rhs,
                        start=(kh == 0),
                        stop=False,
                    )
                for kh in range(5):
                    rhs = bass.AP(
                        tensor=h1r[:].tensor,
                        offset=h1r[:].offset + goff + 12 * kh + 4,
                        ap=[[4 * H1F + 8, 32], [144, 8], [12, 8], [1, 8]],
                    )
                    nc.tensor.matmul(
                        out=ps2[:],
                        lhsT=w2b[:, 64 * kh : 64 * kh + 64],
                        rhs=rhs,
                        start=False,
                        stop=(kh == 4),
                    )
                # pool conv2 8x8 -> 4x4 in two stages
                st1 = S.tile([64, 256], F32, tag="st1")
                iv = ps2[:].rearrange("p (soh pw s0) -> p soh pw s0", pw=4, s0=2)
                nc.vector.tensor_reduce(out=st1[:], in_=iv, axis=AX, op=MAXOP)
                # st1 free = (s, oh, pw): flat = s*32 + (2ph+s1)*4 + pw
                b0 = blk * BLK + g * 8
                outv = bass.AP(
                    tensor=h2[:].tensor,
                    offset=h2[:].offset + b0,
                    ap=[[16 * BPC, 64], [4 * BPC, 4], [BPC, 4], [1, 8]],
                )
                ia = bass.AP(
                    tensor=st1[:].tensor,
                    offset=st1[:].offset,
                    ap=[[256, 64], [8, 4], [1, 4], [32, 8]],
                )
                ib = bass.AP(
                    tensor=st1[:].tensor,
                    offset=st1[:].offset + 4,
                    ap=[[256, 64], [8, 4], [1, 4], [32, 8]],
                )
                nc.vector.tensor_tensor(out=outv, in0=ia, in1=ib, op=MAXOP)

        # ---- h2 bias + relu ----
        nc.scalar.activation(h2[:], h2[:], RELU, bias=b2[:])

        # ---- fc1: relayout h2 -> 8 K-chunks [128, BPC] ----
        hr = []
        for k in range(8):
            t = P.tile([128, BPC + 8], BF16, tag=f"hr{k}")
            src = bass.AP(
                tensor=h2[:].tensor,
                offset=h2[:].offset + 8 * k * 16 * BPC,
                ap=[[16 * BPC, 8], [BPC, 16], [1, BPC]],
            )
            nc.sync.dma_start(out=t[:, 0:BPC], in_=src)
            hr.append(t)

        h3 = []
        for m in range(4):
            psf = PS.tile([128, 512], F32, tag="ps")
            for k in range(8):
                nc.tensor.matmul(
                    out=psf[:],
                    lhsT=w3[:, (k * 4 + m) * 128 : (k * 4 + m) * 128 + 128],
                    rhs=hr[k][:, 0:BPC],
                    start=(k == 0),
                    stop=(k == 7),
                )
            t = P.tile([128, BPC], F32, tag=f"h3{m}")
            nc.scalar.activation(t[:], psf[:], RELU, bias=b3[:, m : m + 1])
            h3.append(t)

        # ---- fc2 + log_softmax, batch on partitions ----
        for bc in range(4):
            psl = PS.tile([128, 10], F32, tag="ps")
            for k in range(4):
                nc.tensor.matmul(
                    out=psl[:],
                    lhsT=h3[k][:, bc * 128 : bc * 128 + 128],
                    rhs=fc2[:, k * 10 : k * 10 + 10],
                    start=(k == 0),
                    stop=False,
                )
            nc.tensor.matmul(
                out=psl[:],
                lhsT=ones1[:],
                rhs=fc2b[:],
                start=False,
                stop=True,
            )
            negm = S.tile([128, 1], F32, tag="negm")
            nc.vector.tensor_reduce(
                out=negm[:], in_=psl[:], axis=AX, op=MAXOP, negate=True
            )
            shifted = S.tile([128, 10], F32, tag="shifted")
            nc.vector.tensor_scalar(
                out=shifted[:], in0=psl[:], scalar1=negm[:], scalar2=None, op0=ADDOP
            )
            ex = S.tile([128, 10], F32, tag="ex")
            se = S.tile([128, 1], F32, tag="se")
            nc.scalar.activation(ex[:], shifted[:], EXP, accum_out=se[:])
            lse = S.tile([128, 1], F32, tag="lse")
            nc.scalar.activation(lse[:], se[:], LN)
            osb = S.tile([128, 10], F32, tag="osb")
            nc.vector.tensor_scalar(
                out=osb[:], in0=shifted[:], scalar1=lse[:], scalar2=None, op0=SUBOP
            )
            nc.sync.dma_start(out=ot[bc * 128 : bc * 128 + 128, :], in_=osb[:])

    nc.finalize()
    return nc


def _prep_weights(inputs):
    """Host-side: densify sketch weights and lay them out for the kernel."""
    h1, h2i, h3i = inputs["hash_idx1"], inputs["hash_idx2"], inputs["hash_idx3"]
    s1, s2, s3 = inputs["sgn1"], inputs["sgn2"], inputs["sgn3"]
    w1, w2, w3 = inputs["w1"], inputs["w2"], inputs["w3"]
    b1, b2, b3 = inputs["b1"], inputs["b2"], inputs["b3"]
    fc2w, fc2b = inputs["fc2_w"], inputs["fc2_b"]

    wc1 = (w1[:, h1] * s1[None, :]).astype(np.float32)            # (32, 25)
    wc2 = (w2[:, h2i] * s2[None, :]).astype(np.float32).reshape(64, 32, 5, 5)
    W3 = (w3[:, h3i] * s3[None, :]).astype(np.float32)            # (512, 1024)

    wc1bd = np.zeros((100, 128), np.float32)
    for j in range(4):
        wc1bd[25 * j : 25 * j + 25, 32 * j : 32 * j + 32] = wc1.T
    # conv2 pass A: lhsT rows (c=kw-copy, ci), cols co; tap (kh, kw=c)
    w2a = np.zeros((128, 5, 64), np.float32)
    for c in range(4):
        for kh in range(5):
            w2a[32 * c : 32 * c + 32, kh, :] = wc2[:, :, kh, c].T
    w2a = w2a.reshape(128, 320)
    # conv2 pass B: kw=4 taps via copy 0
    w2b = np.zeros((32, 5, 64), np.float32)
    for kh in range(5):
        w2b[:, kh, :] = wc2[:, :, kh, 4].T
    w2b = w2b.reshape(32, 320)

    # fc1: lhsT chunk (k,m) = W3.T[128k:128k+128, 128m:128m+128]
    w3sb = np.zeros((128, 8, 4, 128), np.float32)
    W3T = np.ascontiguousarray(W3.T)  # (1024, 512)
    for k in range(8):
        for m in range(4):
            w3sb[:, k, m, :] = W3T[128 * k : 128 * k + 128, 128 * m : 128 * m + 128]
    w3sb = w3sb.reshape(128, 4096)

    fc2sb = np.zeros((128, 4, 10), np.float32)
    for k in range(4):
        fc2sb[:, k, :] = fc2w[:, 128 * k : 128 * k + 128].T
    fc2sb = fc2sb.reshape(128, 40)

    b1r = np.tile(np.asarray(b1, np.float32), 4).reshape(128, 1)
    b3sb = np.asarray(b3, np.float32).reshape(4, 128).T.copy()

    bf = lambda a: np.asarray(a, dtype=ml_dtypes.bfloat16)
    f = lambda a: np.ascontiguousarray(a, dtype=np.float32)
    return {
        "wc1bd": bf(wc1bd),
        "w2a": bf(w2a),
        "w2b": bf(w2b),
        "w3sb": bf(w3sb),
        "fc2sb": f(fc2sb),
        "b1r": f(b1r),
        "b2": f(np.asarray(b2).reshape(64, 1)),
        "b3sb": f(b3sb),
        "fc2b": f(np.asarray(fc2b).reshape(1, 10)),
    }


def kernel(**inputs):
    out, _ = _run(inputs, trace=False)
    return out


def _run(inputs, trace=False, tmpdir=None):
    if "nc" not in _CACHE:
        _CACHE["nc"] = _build()
    nc = _CACHE["nc"]

    wmap = _prep_weights(inputs)
    x = np.asarray(inputs["x"], np.float32).reshape(4096, 784)

    in_maps = []
    for c in range(NCORES):
        xs = x[c * BPC : (c + 1) * BPC].reshape(-1)
        xs = np.concatenate([xs, np.zeros(XPAD, np.float32)])
        m = dict(wmap)
        m["x"] = np.asarray(xs, dtype=ml_dtypes.bfloat16)
        in_maps.append(m)

    res = run_bass_kernel_spmd(
        nc, in_maps, core_ids=list(range(NCORES)), trace=trace, tmpdir=tmpdir
    )
    out = np.concatenate([res.results[c]["out"] for c in range(NCORES)], axis=0)
    return out.astype(np.float32), res

